# revision 22
# baseline (speedup 1.0000x reference)
"""MinGRU Trainium2 kernel.

Math (linear-space reformulation of the reference's log-space scan; all
quantities are positive so this is numerically safe):
    k = x @ W_z.T ; u = x @ W_h.T
    c_t = sigmoid(-k_t)            # decay coeff (1 - z_t)
    z_t = sigmoid(k_t)
    v_t = z_t * g(u_t)             # input term, g(u) = relu(u) + sigmoid(min(u,0))
    h_t = c_t * h_{t-1} + v_t      # hardware tensor_tensor_scan (mult, add)
    h_0 = g(h0)

Sharding: 8 cores = 4 batches x 2 halves of d_model output channels.
Each core: x[b].T [1024,4096], weight slices W.T[:, half] [1024,512],
out h.T [512,4096] fp32.  x and W are fed in fp16 (host-cast): the PE
upconverts both to FP22 exactly as it does for f32r, the quantization
error lands well under the accuracy budget, all input DMA halves, and
fp16 weights enable fast weight load (measured: 234 -> 216 ns/matmul).

Per chunk (TC=512 cols), per e-tile (128 channels):
  PE:     8+8 accumulating MMs into pk{e}/pu{e}
  ACT:    c = sig(-k), z = sig(k), r = relu(-u), s = sig(-r)
  GpSimd: g = max(u,0) + s  (scalar_tensor_tensor)
  DVE:    v = z*g, tensor_tensor_scan(c, v)

Scheduling rules baked in (each one traced to a measured stall):
- Only sync and scalar have hardware DGE queues; gpsimd DMA is software
  and far too slow for bulk data.
- A DMA trigger blocks its engine's instruction queue while the DGE
  descriptor ring is full, so the scalar/ACT queue carries only the
  small chunk-0 x tiles (needed before any ACT work exists) plus h
  outputs deferred one full chunk (scan dependency already satisfied,
  ring drained in between).  All other bulk DMA lives on sync, which
  runs no compute.
- Each e-tile's post chain is laid out so every engine's op sequence is
  self-ordered (ACT: c,z,r,s per e; Pool: g per e; DVE: v,scan per e).
  The static scheduler underestimates scan/Pool costs, so cross-engine
  ping-pong (v on Pool between two DVE ops) serializes e-tiles; this
  assignment has no such bounce.
- Chunk 0 runs kt-outer so PE consumption tracks the kt-granular DMA
  arrival order; its pk-freeing sigmoids are issued before the rest of
  its post chain, and chunk 1 consumes pk banks before pu banks to
  match the order chunk 0 frees them.
- Warmup matmuls keep the PE busy through the HAM activity window (PE
  clock 1.2 -> 2.4 GHz after ~3.4us of sustained activity) until the
  first real operands arrive; dummy activations in the preamble pull
  the ~1.3us ACT table loads off the critical path.
- The final e-tile of the final chunk runs pk first, its pu matmuls in
  column-half groups, and its r/s/g/v/scan in column halves, so the
  serial tail after the very last matmul is a ~2.5us half-width chain;
  final outputs drain per e-tile on the by-then-idle sync queue.
"""

import numpy as np

B, T, D = 4, 4096, 1024
EC = 512            # output channels per core
ET = EC // 128      # 4 e-tiles per core
KT = D // 128       # 8 k-tiles
TC = 512            # time chunk (columns per matmul / PSUM bank)
NCHUNK = T // TC    # 8

_CACHED = {}
LAST_RESULT = None


def _build_nc():
    import concourse.bass as bass
    import concourse.bacc as bacc
    import concourse.mybir as mybir
    import concourse.tile as tile

    f32 = mybir.dt.float32
    f16 = mybir.dt.float16
    AF = mybir.ActivationFunctionType
    OP = mybir.AluOpType

    nc = bacc.Bacc(None, target_bir_lowering=False)

    xT = nc.dram_tensor("xT", [D, T], f16, kind="ExternalInput")
    wz = nc.dram_tensor("wzT", [D, EC], f16, kind="ExternalInput")
    wh = nc.dram_tensor("whT", [D, EC], f16, kind="ExternalInput")
    h0g = nc.dram_tensor("h0g", [128, ET], f32, kind="ExternalInput")
    hT = nc.dram_tensor("hT", [EC, T], f32, kind="ExternalOutput")

    xT_ap = xT[:].rearrange("(kt p) t -> p kt t", p=128)
    wz_ap = wz[:].rearrange("(kt p) e -> p kt e", p=128)
    wh_ap = wh[:].rearrange("(kt p) e -> p kt e", p=128)
    hT_ap = hT[:].rearrange("(et p) t -> p et t", p=128)

    with tile.TileContext(nc) as tcx:
        with (
            tcx.tile_pool(name="wpool", bufs=1) as wpool,
            tcx.tile_pool(name="xpool", bufs=3) as xpool,
            tcx.tile_pool(name="work", bufs=2) as work,
            tcx.tile_pool(name="hpool", bufs=2) as hpool,
            tcx.tile_pool(name="psum", bufs=1, space=bass.MemorySpace.PSUM) as psum,
        ):
            wzk = [wpool.tile([128, EC], f16, tag=f"wz{kt}", name=f"wzk{kt}")
                   for kt in range(KT)]
            whk = [wpool.tile([128, EC], f16, tag=f"wh{kt}", name=f"whk{kt}")
                   for kt in range(KT)]
            h0_sb = wpool.tile([128, ET], f32, tag="h0")
            warm = wpool.tile([128, TC], f16, tag="warm")
            dummy = wpool.tile([128, 2], f32, tag="dummy")
            nc.vector.memset(warm[:], 0.0)
            nc.vector.memset(dummy[:], 0.0)

            # Dummy activations: force the ACT function-table loads
            # (~1.3us each) to happen in the preamble, not in front of the
            # first real chunk's post-processing.
            nc.scalar.activation(dummy[:, 1:2], dummy[:, 0:1],
                                 AF.Sigmoid, scale=-1.0)
            nc.scalar.activation(dummy[:, 1:2], dummy[:, 0:1],
                                 AF.Relu, scale=-1.0)

            # Weights interleaved z/h on sync: chunk 0 consumes them
            # kt-ordered, gate-interleaved, matching arrival order.
            for kt in range(KT):
                nc.sync.dma_start(out=wzk[kt][:], in_=wz_ap[:, kt, :])
                nc.sync.dma_start(out=whk[kt][:], in_=wh_ap[:, kt, :])
            nc.sync.dma_start(out=h0_sb[:], in_=h0g[:])

            # chunk-0 x, kt-granular, on scalar (the only bulk DMA allowed
            # there: it must land before any ACT work exists)
            x0k = [xpool.tile([128, TC], f16, tag=f"x0_{kt}",
                              name=f"x0k{kt}", bufs=1)
                   for kt in range(KT)]
            for kt in range(KT):
                nc.scalar.dma_start(out=x0k[kt][:], in_=xT_ap[:, kt, 0:TC])

            def x_fetch(ci):
                t = xpool.tile([128, KT, TC], f16, tag="x", name=f"x_{ci}")
                # x1 rides scalar (free after the 1 MB of chunk-0 tiles):
                # on sync it would complete behind all 16 weight transfers,
                # stalling chunk 1's first matmul group
                eng = nc.scalar if ci == 1 else nc.sync
                eng.dma_start(out=t[:], in_=xT_ap[:, :, ci * TC:(ci + 1) * TC])
                return t

            xtiles = {1: x_fetch(1), 2: x_fetch(2)}

            h_prev = None
            for ci in range(NCHUNK):
                off = ci * TC
                last = ci == NCHUNK - 1
                if ci == 0:
                    x_sb = None
                    xsl = lambda kt, a=0, b=TC: x0k[kt][:, a:b]
                else:
                    x_sb = xtiles.pop(ci)
                    xsl = (lambda kt, a=0, b=TC, _x=x_sb: _x[:, kt, a:b])
                xc = [xsl(kt) for kt in range(KT)]
                if ci + 2 < NCHUNK and (ci + 2) not in xtiles:
                    xtiles[ci + 2] = x_fetch(ci + 2)

                pk = [psum.tile([128, TC], f32, tag=f"pk{e}", name=f"pk{e}_{ci}")
                      for e in range(ET)]
                pu = [psum.tile([128, TC], f32, tag=f"pu{e}", name=f"pu{e}_{ci}")
                      for e in range(ET)]

                def mm_group(dst, wk, e, a, b):
                    esl = slice(e * 128, (e + 1) * 128)
                    for kt in range(KT):
                        nc.tensor.matmul(dst[:, a:b], wk[kt][:, esl],
                                         xsl(kt, a, b),
                                         start=(kt == 0), stop=(kt == KT - 1))

                if ci == 0:
                    # HAM warmups: prime the PE through the ~3.4us activity
                    # window (1.2 -> 2.4 GHz) while the first DMAs land, so
                    # every real matmul runs warm.  (Tested without: the
                    # first real matmuls then run at the 0.65-0.8 GHz cold
                    # pstate and lose more than the warmups cost.)
                    for _ in range(8):
                        nc.tensor.matmul(pk[0][:], warm[:, 0:128], warm[:],
                                         start=True, stop=True)
                    # kt-outer: consumption tracks the DMA arrival order
                    for kt in range(KT):
                        for e in range(ET):
                            esl = slice(e * 128, (e + 1) * 128)
                            nc.tensor.matmul(pk[e][:], wzk[kt][:, esl], xc[kt],
                                             start=(kt == 0), stop=(kt == KT - 1))
                            nc.tensor.matmul(pu[e][:], whk[kt][:, esl], xc[kt],
                                             start=(kt == 0), stop=(kt == KT - 1))
                elif ci == 1:
                    # chunk 0's post all lands after its last matmul
                    # (kt-outer): consume the banks it frees first (pk,
                    # freed by the quick sigmoids) before the pu banks.
                    for dst, wk in [(pk, wzk), (pu, whk)]:
                        for e in range(ET):
                            mm_group(dst[e], wk, e, 0, TC)
                elif not last:
                    for e in range(ET):
                        mm_group(pk[e], wzk, e, 0, TC)
                        mm_group(pu[e], whk, e, 0, TC)
                else:
                    # final chunk: pu-first everywhere so the r/s/g/mv
                    # chains (which need pu) run during later matmuls; e3's
                    # pk goes dead last, in column-half groups, leaving
                    # only c3 + scan3 (half-width) after the final matmul.
                    for e in range(ET - 1):
                        mm_group(pu[e], whk, e, 0, TC)
                        mm_group(pk[e], wzk, e, 0, TC)
                    e = ET - 1
                    mm_group(pu[e], whk, e, 0, TC)
                    mm_group(pk[e], wzk, e, 0, TC // 2)
                    mm_group(pk[e], wzk, e, TC // 2, TC)

                # previous chunk's output, deferred one full chunk so the
                # trigger's scan dependency is already satisfied and never
                # head-blocks the scalar queue
                if ci >= 1:
                    poff = (ci - 1) * TC
                    nc.scalar.dma_start(out=hT_ap[:, :, poff:poff + TC],
                                        in_=h_prev[:])

                h = hpool.tile([128, ET, TC], f32, tag="h", name=f"h_{ci}")

                wt = []
                for e in range(ET):
                    c = work.tile([128, TC], f32, tag=f"c{e}", name=f"c{e}_{ci}")
                    r = work.tile([128, TC], f32, tag=f"r{e}", name=f"r{e}_{ci}")
                    s = work.tile([128, TC], f32, tag=f"s{e}", name=f"s{e}_{ci}")
                    # r is dead once s = sig(-r) is computed, and s is dead
                    # once g = relu(u)+s is; reuse their rings to fit SBUF
                    g = work.tile([128, TC], f32, tag=f"r{e}", name=f"g{e}_{ci}")
                    v = work.tile([128, TC], f32, tag=f"s{e}", name=f"v{e}_{ci}")
                    wt.append((c, r, s, g, v))

                # column spans for c/mv/scan: the final e-tile of the final
                # chunk runs in halves (its pk groups are also halved), so
                # only c3b -> mv3b -> scan3b trail the very last matmul
                def spans(e):
                    if last and e == ET - 1:
                        return [(0, TC // 2), (TC // 2, TC)]
                    return [(0, TC)]

                # ACT: c frees the pk bank; chunk 0 issues all four c's
                # first (every pk lands at once there).  For the final
                # e-tile r/s go first (its pu group precedes its pk).
                if ci == 0:
                    for e in range(ET):
                        nc.scalar.activation(wt[e][0][:], pk[e][:],
                                             AF.Sigmoid, scale=-1.0)
                for e in range(ET):
                    c, r, s, g, v = wt[e]
                    e_last = last and e == ET - 1
                    if not e_last and ci != 0:
                        nc.scalar.activation(c[:], pk[e][:],
                                             AF.Sigmoid, scale=-1.0)
                    nc.scalar.activation(r[:], pu[e][:], AF.Relu, scale=-1.0)
                    nc.scalar.activation(s[:], r[:], AF.Sigmoid, scale=-1.0)
                    if e_last:
                        for (a, b) in spans(e):
                            nc.scalar.activation(c[:, a:b], pk[e][:, a:b],
                                                 AF.Sigmoid, scale=-1.0)
                # DVE: g (frees the pu bank), mv = (c-1)*g = -z*g, then
                # h = c*h_prev - mv -- a fully intra-engine chain, immune
                # to scheduler order choices, with no z intermediate
                for e in range(ET):
                    c, r, s, g, v = wt[e]
                    nc.vector.scalar_tensor_tensor(g[:], pu[e][:], 0.0, s[:],
                                                   op0=OP.max, op1=OP.add)
                    for (a, b) in spans(e):
                        cs = slice(a, b)
                        nc.vector.scalar_tensor_tensor(v[:, cs], c[:, cs],
                                                       1.0, g[:, cs],
                                                       op0=OP.subtract,
                                                       op1=OP.mult)
                        if a == 0:
                            init = (h0_sb[:, e:e + 1] if ci == 0
                                    else h_prev[:, e, TC - 1:TC])
                        else:
                            init = h[:, e, a - 1:a]
                        nc.vector.tensor_tensor_scan(h[:, e, a:b], c[:, cs],
                                                     v[:, cs], init,
                                                     op0=OP.mult,
                                                     op1=OP.subtract)
                if last:
                    # per-piece drain, alternating the two hardware queues
                    n = 0
                    for e in range(ET):
                        for (a, b) in spans(e):
                            eng = nc.sync if n % 2 == 0 else nc.scalar
                            eng.dma_start(out=hT_ap[:, e, off + a:off + b],
                                          in_=h[:, e, a:b])
                            n += 1
                h_prev = h

    nc.compile()
    return nc


def _get_nc():
    if "nc" not in _CACHED:
        _CACHED["nc"] = _build_nc()
    return _CACHED["nc"]


def kernel(x, h0, W_h, W_z, _trace=False):
    global LAST_RESULT
    from concourse import bass_utils

    x = np.asarray(x, np.float32)
    h0 = np.asarray(h0, np.float32)
    W_h = np.asarray(W_h, np.float32)
    W_z = np.asarray(W_z, np.float32)

    # host-side prep: transposes, fp16 casts, initial state g(h0)
    gh0 = np.where(h0 >= 0, h0 + np.float32(0.5),
                   1.0 / (1.0 + np.exp(-h0))).astype(np.float32)  # [B,1,D]
    WzT = np.ascontiguousarray(W_z.T.astype(np.float16))  # [D, D]
    WhT = np.ascontiguousarray(W_h.T.astype(np.float16))

    in_maps = []
    for b in range(B):
        xTb = np.ascontiguousarray(x[b].T.astype(np.float16))  # [D, T]
        for eh in range(2):
            esl = slice(eh * EC, (eh + 1) * EC)
            h0c = np.ascontiguousarray(
                gh0[b, 0, esl].reshape(ET, 128).T)  # [128, ET]
            in_maps.append({
                "xT": xTb,
                "wzT": np.ascontiguousarray(WzT[:, esl]),
                "whT": np.ascontiguousarray(WhT[:, esl]),
                "h0g": h0c,
            })

    nc = _get_nc()
    try:
        res = bass_utils.run_bass_kernel_spmd(
            nc, in_maps, core_ids=list(range(8)), trace=_trace,
        )
    except Exception:
        # transient NRT_EXEC_UNIT_UNRECOVERABLE has been observed on a
        # first execution; one retry has always succeeded
        res = bass_utils.run_bass_kernel_spmd(
            nc, in_maps, core_ids=list(range(8)), trace=_trace,
        )
    LAST_RESULT = res

    out = np.empty((B, T, D), np.float32)
    for b in range(B):
        for eh in range(2):
            core = b * 2 + eh
            out[b, :, eh * EC:(eh + 1) * EC] = res.results[core]["hT"].T
    return out
